# revision 4
# baseline (speedup 1.0000x reference)
"""Low-rank attention kernel for Trainium2, distributed over 8 NeuronCores.

Math (per batch b):
    u  = q @ Wu            [N, R]
    vp = k @ Wv            [N, R]
    S  = u @ vp.T / sqrt(R)
    out = softmax(S) @ v   [N, D]

Shapes: B=4, N=4096, D=1024, R=32.

Sharding: data-parallel over batch x row-halves -> 8 shards. Core c handles
batch b = c // 2, rows [h*2048, (h+1)*2048) with h = c % 2. Each core gets its
q-shard and the full k/v for its batch. q/k are fed pre-transposed ([D, n]
layout) so every matmul contraction lands on the partition axis with no
on-device transposes.

Per-core device kernel (all matmuls in float32r: full PE rate, ~1e-4 rel err):
  1. uT[R, 2048]  = sum_d Wu[d, :].T qT[d, :]   (K=128 d-tiles, PSUM accum)
     vpT[R, 4096] = sum_d Wv[d, :].T kT[d, :]
  2. flash-style main loop over n-chunks of 256 rows:
       for each m-tile (128 cols): scoresT[m128, n256] = vpT_tile.T @ uT_chunk
       expT = Exp(scoresT / sqrt(R))                       (ScalarE, PSUM->SBUF)
       out_acc[n128, d512] += expT_tile.T @ v_tile         (PSUM accum over m)
       sum_acc[n128, 1]    += expT_tile.T @ ones
     out = out_acc * (1 / sum_acc)   (softmax normalization folded at the end)
"""

import numpy as np

B, N, D, R = 4, 4096, 1024, 32
NLOC = N // 2            # rows per core
RSCALE = float(1.0 / np.sqrt(np.float32(R)))

N_CHUNK = 256            # rows of scores computed per PSUM round
M_TILE = 128             # contraction tile for the AV matmul
D_HALF = 512             # PSUM bank width in fp32

LAST_RESULT = None       # test.py reads exec_time_ns etc. from here


def _build():
    from concourse import bacc, mybir
    from concourse.tile import TileContext

    f32 = mybir.dt.float32
    f32r = mybir.dt.float32r
    EXP = mybir.ActivationFunctionType.Exp
    COPY = mybir.ActivationFunctionType.Copy

    nc = bacc.Bacc("TRN2", target_bir_lowering=False)

    qT = nc.dram_tensor("qT", [D, NLOC], f32r, kind="ExternalInput")
    kT = nc.dram_tensor("kT", [D, N], f32r, kind="ExternalInput")
    v = nc.dram_tensor("v", [N, D], f32r, kind="ExternalInput")
    wu = nc.dram_tensor("wu", [D, R], f32r, kind="ExternalInput")
    wv = nc.dram_tensor("wv", [D, R], f32r, kind="ExternalInput")
    o = nc.dram_tensor("o", [NLOC, D], f32, kind="ExternalOutput")

    DT = D // 128         # 8 d-tiles
    NQ = NLOC // 1024     # 2 column-halves of qT
    MQ = N // 1024        # 4 column-quarters of kT
    NCH = NLOC // N_CHUNK  # 8 main-loop chunks
    MT = N // M_TILE      # 32 m tiles
    VG = 8                # v row-groups of 512
    VPG = N // VG // 128  # 4 m-tiles per v group

    with TileContext(nc) as tc:
        with tc.tile_pool(name="singles", bufs=1) as singles, \
             tc.tile_pool(name="stream", bufs=10) as stream, \
             tc.tile_pool(name="vpool", bufs=VG) as vpool, \
             tc.tile_pool(name="expp", bufs=3) as expp, \
             tc.tile_pool(name="outp", bufs=2) as outp, \
             tc.tile_pool(name="rpool", bufs=4) as rpool, \
             tc.tile_pool(name="pacc", bufs=4, space="PSUM") as pacc, \
             tc.tile_pool(name="pscore", bufs=2, space="PSUM") as pscore, \
             tc.tile_pool(name="psums", bufs=2, space="PSUM") as psums:

            # ---- constants / projection weights ----
            wu_sb = singles.tile([128, DT, R], f32r, tag="wu")
            nc.sync.dma_start(out=wu_sb, in_=wu.rearrange("(t p) r -> p t r", p=128))
            wv_sb = singles.tile([128, DT, R], f32r, tag="wv")
            nc.sync.dma_start(out=wv_sb, in_=wv.rearrange("(t p) r -> p t r", p=128))
            ones_f = singles.tile([128, 2], f32, tag="ones_f")
            nc.vector.memset(ones_f, 1.0)
            ones = singles.tile([128, 2], f32r, tag="ones")
            nc.vector.tensor_copy(out=ones, in_=ones_f)

            uT = singles.tile([R, NLOC], f32r, tag="uT")
            vpT = singles.tile([R, N], f32r, tag="vpT")

            # ---- persistent v tiles: v_sb[g][p, t, d] = v[g*512 + t*128 + p, d]
            v_sb = []
            for g in range(VG):
                vt = vpool.tile([128, VPG, D], f32r, tag="v")
                nc.sync.dma_start(
                    out=vt, in_=v[g * 512:(g + 1) * 512, :].rearrange(
                        "(t p) d -> p t d", p=128))
                v_sb.append(vt)

            # ---- phase 1a: uT = Wu.T @ q  (per d-tile: wu_sb[:,t,:].T @ qT_t)
            qt = {}
            for h in range(NQ):
                for t in range(DT):
                    tile = stream.tile([128, 1024], f32r, tag="stream")
                    nc.sync.dma_start(
                        out=tile, in_=qT[t * 128:(t + 1) * 128,
                                         h * 1024:(h + 1) * 1024])
                    qt[(t, h)] = tile
            for c in range(NLOC // 512):
                h, off = c // 2, (c % 2) * 512
                pu = pscore.tile([R, 512], f32, tag="scores")
                for t in range(DT):
                    nc.tensor.matmul(pu, lhsT=wu_sb[:, t, :],
                                     rhs=qt[(t, h)][:, off:off + 512],
                                     start=(t == 0), stop=(t == DT - 1))
                nc.vector.tensor_copy(out=uT[:, c * 512:(c + 1) * 512], in_=pu)

            # ---- phase 1b: vpT = Wv.T @ k
            for qtr in range(MQ):
                kt = []
                for t in range(DT):
                    tile = stream.tile([128, 1024], f32r, tag="stream")
                    nc.sync.dma_start(
                        out=tile, in_=kT[t * 128:(t + 1) * 128,
                                         qtr * 1024:(qtr + 1) * 1024])
                    kt.append(tile)
                for c2 in range(2):
                    pv = pscore.tile([R, 512], f32, tag="scores")
                    for t in range(DT):
                        nc.tensor.matmul(pv, lhsT=wv_sb[:, t, :],
                                         rhs=kt[t][:, c2 * 512:c2 * 512 + 512],
                                         start=(t == 0), stop=(t == DT - 1))
                    off = qtr * 1024 + c2 * 512
                    nc.vector.tensor_copy(out=vpT[:, off:off + 512], in_=pv)

            # ---- phase 2: flash-style scores/softmax/AV ----
            for ch in range(NCH):
                accs = [pacc.tile([128, D_HALF], f32, tag="acc", name=f"acc{ch}_{i}")
                        for i in range(4)]
                sums = [psums.tile([128, 2], f32, tag="sums", name=f"sum{ch}_{i}") for i in range(2)]
                for mt in range(MT):
                    g, tg = mt // VPG, mt % VPG
                    ps = pscore.tile([128, N_CHUNK], f32, tag="scores")
                    nc.tensor.matmul(
                        ps, lhsT=vpT[:, mt * 128:(mt + 1) * 128],
                        rhs=uT[:, ch * N_CHUNK:(ch + 1) * N_CHUNK],
                        start=True, stop=True)
                    ex = expp.tile([128, N_CHUNK], f32r, tag="ex")
                    nc.scalar.activation(out=ex, in_=ps, func=EXP, scale=RSCALE)
                    first, last = (mt == 0), (mt == MT - 1)
                    for j in range(2):
                        lhs = ex[:, j * 128:(j + 1) * 128]
                        nc.tensor.matmul(accs[2 * j], lhsT=lhs,
                                         rhs=v_sb[g][:, tg, 0:D_HALF],
                                         start=first, stop=last)
                        nc.tensor.matmul(accs[2 * j + 1], lhsT=lhs,
                                         rhs=v_sb[g][:, tg, D_HALF:D],
                                         start=first, stop=last)
                        nc.tensor.matmul(sums[j], lhsT=lhs, rhs=ones,
                                         start=first, stop=last)
                # normalize + store 2 row-tiles of 128
                for j in range(2):
                    rc = rpool.tile([128, 1], f32, tag="rc")
                    nc.vector.reciprocal(rc, sums[j][:, 0:1])
                    ob = outp.tile([128, D], f32, tag="ob")
                    nc.scalar.activation(out=ob[:, 0:D_HALF], in_=accs[2 * j],
                                         func=COPY, scale=rc)
                    nc.scalar.activation(out=ob[:, D_HALF:D],
                                         in_=accs[2 * j + 1], func=COPY,
                                         scale=rc)
                    row = ch * N_CHUNK + j * 128
                    nc.sync.dma_start(out=o[row:row + 128, :], in_=ob)

    nc.finalize()
    return nc


def kernel(q, k, v, Wu, Wv):
    global LAST_RESULT
    from concourse import bass_utils

    nc = _build()

    kTs = [np.ascontiguousarray(k[b].T) for b in range(B)]
    in_maps = []
    for core in range(8):
        b, h = core // 2, core % 2
        in_maps.append({
            "qT": np.ascontiguousarray(q[b].T[:, h * NLOC:(h + 1) * NLOC]),
            "kT": kTs[b],
            "v": np.ascontiguousarray(v[b]),
            "wu": np.ascontiguousarray(Wu),
            "wv": np.ascontiguousarray(Wv),
        })

    res = bass_utils.run_bass_kernel_spmd(nc, in_maps, core_ids=list(range(8)))
    LAST_RESULT = res

    out = np.empty((B, N, D), dtype=np.float32)
    for core in range(8):
        b, h = core // 2, core % 2
        out[b, h * NLOC:(h + 1) * NLOC, :] = res.results[core]["o"]
    return out


# revision 9
# speedup vs baseline: 1.3403x; 1.3403x over previous
"""Low-rank attention kernel for Trainium2, distributed over 8 NeuronCores.

Math (per batch b):
    u  = q @ Wu            [N, R]
    vp = k @ Wv            [N, R]
    S  = u @ vp.T / sqrt(R)
    out = softmax(S) @ v   [N, D]

Shapes: B=4, N=4096, D=1024, R=32.

Sharding: data-parallel over batch x row-halves -> 8 shards. Core c handles
batch b = c // 2, rows [h*2048, (h+1)*2048) with h = c % 2. Each core gets its
q-shard and the full k/v for its batch. q/k are fed pre-transposed ([D, n]
layout) so every matmul contraction lands on the partition axis with no
on-device transposes.

Per-core device kernel (all matmuls in float32r: full PE rate, ~1e-4 rel err):
  1. uT[R, 2048]  = sum_d Wu[d, :].T qT[d, :]   (K=128 d-tiles, PSUM accum)
     vpT[R, 4096] = sum_d Wv[d, :].T kT[d, :]
  2. flash-style main loop over n-chunks of 256 rows:
       for each m-tile (128 cols): scoresT[m128, n256] = vpT_tile.T @ uT_chunk
       expT = Exp(scoresT / sqrt(R))                       (ScalarE, PSUM->SBUF)
       out_acc[n128, d512] += expT_tile.T @ v_tile         (PSUM accum over m)
       sum_acc[n128, 1]    += expT_tile.T @ ones
     out = out_acc * (1 / sum_acc)   (softmax normalization folded at the end)
"""

import numpy as np

B, N, D, R = 4, 4096, 1024, 32
NLOC = N // 2            # rows per core
RSCALE = float(1.0 / np.sqrt(np.float32(R)))

N_CHUNK = 256            # rows of scores computed per PSUM round
M_TILE = 128             # contraction tile for the AV matmul
D_HALF = 512             # PSUM bank width in fp32

LAST_RESULT = None       # test.py reads exec_time_ns etc. from here


def _build():
    from concourse import bacc, mybir
    from concourse.tile import TileContext

    f32 = mybir.dt.float32
    f32r = mybir.dt.float32r
    f16 = mybir.dt.float16
    EXP = mybir.ActivationFunctionType.Exp
    COPY = mybir.ActivationFunctionType.Copy

    nc = bacc.Bacc("TRN2", target_bir_lowering=False)

    qT = nc.dram_tensor("qT", [D, NLOC], f32r, kind="ExternalInput")
    kT = nc.dram_tensor("kT", [D, N], f32r, kind="ExternalInput")
    v = nc.dram_tensor("v", [N, D], f16, kind="ExternalInput")
    wu = nc.dram_tensor("wu", [D, R], f32r, kind="ExternalInput")
    wv = nc.dram_tensor("wv", [D, R], f32r, kind="ExternalInput")
    o = nc.dram_tensor("o", [NLOC, D], f32, kind="ExternalOutput")

    DT = D // 128         # 8 d-tiles
    NQ = NLOC // 1024     # 2 column-halves of qT
    MQ = N // 1024        # 4 column-quarters of kT
    NCH = NLOC // N_CHUNK  # 8 main-loop chunks
    MT = N // M_TILE      # 32 m tiles
    VG = 8                # v row-groups of 512
    VPG = N // VG // 128  # 4 m-tiles per v group

    with TileContext(nc) as tc:
        with tc.tile_pool(name="singles", bufs=1) as singles, \
             tc.tile_pool(name="stream", bufs=20) as stream, \
             tc.tile_pool(name="vpool", bufs=VG) as vpool, \
             tc.tile_pool(name="expp", bufs=4) as expp, \
             tc.tile_pool(name="outp", bufs=3) as outp, \
             tc.tile_pool(name="rpool", bufs=4) as rpool, \
             tc.tile_pool(name="pacc", bufs=4, space="PSUM") as pacc, \
             tc.tile_pool(name="pscore", bufs=2, space="PSUM") as pscore, \
             tc.tile_pool(name="psums", bufs=2, space="PSUM") as psums:

            # ---- constants / projection weights ----
            wu_sb = singles.tile([128, DT, R], f32r, tag="wu")
            nc.sync.dma_start(out=wu_sb, in_=wu.rearrange("(t p) r -> p t r", p=128))
            wv_sb = singles.tile([128, DT, R], f32r, tag="wv")
            nc.sync.dma_start(out=wv_sb, in_=wv.rearrange("(t p) r -> p t r", p=128))
            ones = singles.tile([128, 2], f16, tag="ones")
            nc.vector.memset(ones, 1.0)

            uT = singles.tile([R, NLOC], f32r, tag="uT")
            vpT = singles.tile([R, N], f32r, tag="vpT")

            # ---- phase 1a: uT = Wu.T @ q  (per d-tile: wu_sb[:,t,:].T @ qT_t)
            def load_qt(h):
                tiles = []
                for t in range(DT):
                    tile = stream.tile([128, 1024], f32r, tag="stream",
                                       name=f"qt{h}_{t}")
                    nc.sync.dma_start(
                        out=tile, in_=qT[t * 128:(t + 1) * 128,
                                         h * 1024:(h + 1) * 1024])
                    tiles.append(tile)
                return tiles

            qt = {}
            for t, tile in enumerate(load_qt(0)):
                qt[(t, 0)] = tile
            def u_chunk(c):
                h, off = c // 2, (c % 2) * 512
                pu = pscore.tile([R, 512], f32, tag="scores", name=f"pu{c}")
                for t in range(DT):
                    nc.tensor.matmul(pu, lhsT=wu_sb[:, t, :],
                                     rhs=qt[(t, h)][:, off:off + 512],
                                     start=(t == 0), stop=(t == DT - 1))
                nc.vector.tensor_copy(out=uT[:, c * 512:(c + 1) * 512], in_=pu)

            for c in (0, 1):
                u_chunk(c)

            # ---- v tiles, interleaved with kT quarters so neither starves
            v_sb = [None] * VG

            def load_v(g):
                vt = vpool.tile([128, VPG, D], f16, tag="v", name=f"v{g}")
                nc.sync.dma_start(
                    out=vt, in_=v[g * 512:(g + 1) * 512, :].rearrange(
                        "(t p) d -> p t d", p=128))
                v_sb[g] = vt

            load_v(0)
            load_v(1)

            # ---- phase 1b: vpT = Wv.T @ k
            for qtr in range(MQ):
                kt = []
                for t in range(DT):
                    tile = stream.tile([128, 1024], f32r, tag="stream")
                    nc.sync.dma_start(
                        out=tile, in_=kT[t * 128:(t + 1) * 128,
                                         qtr * 1024:(qtr + 1) * 1024])
                    kt.append(tile)
                if qtr < 3:
                    load_v(2 + 2 * qtr)
                    load_v(3 + 2 * qtr)
                for c2 in range(2):
                    pv = pscore.tile([R, 512], f32, tag="scores")
                    for t in range(DT):
                        nc.tensor.matmul(pv, lhsT=wv_sb[:, t, :],
                                         rhs=kt[t][:, c2 * 512:c2 * 512 + 512],
                                         start=(t == 0), stop=(t == DT - 1))
                    off = qtr * 1024 + c2 * 512
                    nc.vector.tensor_copy(out=vpT[:, off:off + 512], in_=pv)

            for t, tile in enumerate(load_qt(1)):
                qt[(t, 1)] = tile
            for c in (2, 3):
                u_chunk(c)

            # ---- phase 2: flash-style scores/softmax/AV ----
            # software-pipelined: scores/exp for m-tile mt+1 are issued before
            # the AV matmuls of m-tile mt, so ScalarE exp latency hides under
            # the previous tile's AV work on the PE.
            for ch in range(NCH):
                accs = [pacc.tile([128, D_HALF], f32, tag="acc", name=f"acc{ch}_{i}")
                        for i in range(4)]
                # one PSUM bank per accumulator: start=True clears has_written
                # bank-wide, so co-locating two accumulation groups in one bank
                # silently drops the first group's first contribution
                sums = [psums.tile([128, 2], f32, tag="sums", name=f"sum{ch}_{i}")
                        for i in range(2)]

                def scores_exp(mt):
                    ps = pscore.tile([128, N_CHUNK], f32, tag="scores",
                                     name=f"ps{ch}_{mt}")
                    nc.tensor.matmul(
                        ps, lhsT=vpT[:, mt * 128:(mt + 1) * 128],
                        rhs=uT[:, ch * N_CHUNK:(ch + 1) * N_CHUNK],
                        start=True, stop=True)
                    ex = expp.tile([128, N_CHUNK], f16, tag="ex",
                                   name=f"ex{ch}_{mt}")
                    nc.scalar.activation(out=ex, in_=ps, func=EXP, scale=RSCALE)
                    return ex

                ex_prev = scores_exp(0)
                for mt in range(MT):
                    ex = ex_prev
                    if mt + 1 < MT:
                        ex_prev = scores_exp(mt + 1)
                    g, tg = mt // VPG, mt % VPG
                    first, last = (mt == 0), (mt == MT - 1)
                    for j in range(2):
                        lhs = ex[:, j * 128:(j + 1) * 128]
                        nc.tensor.matmul(accs[2 * j], lhsT=lhs,
                                         rhs=v_sb[g][:, tg, 0:D_HALF],
                                         start=first, stop=last)
                        nc.tensor.matmul(accs[2 * j + 1], lhsT=lhs,
                                         rhs=v_sb[g][:, tg, D_HALF:D],
                                         start=first, stop=last)
                        nc.tensor.matmul(sums[j], lhsT=lhs, rhs=ones,
                                         start=first, stop=last)
                # normalize on DVE (keeps ScalarE free for next chunk's exp)
                for j in range(2):
                    rc = rpool.tile([128, 1], f32, tag="rc", name=f"rc{ch}_{j}")
                    nc.vector.reciprocal(rc, sums[j][:, 0:1])
                    ob = outp.tile([128, D], f32, tag="ob", name=f"ob{ch}_{j}")
                    nc.vector.tensor_scalar_mul(ob[:, 0:D_HALF], accs[2 * j], rc)
                    nc.vector.tensor_scalar_mul(ob[:, D_HALF:D], accs[2 * j + 1], rc)
                    row = ch * N_CHUNK + j * 128
                    nc.sync.dma_start(out=o[row:row + 128, :], in_=ob)

    nc.finalize()
    return nc


def kernel(q, k, v, Wu, Wv):
    global LAST_RESULT
    from concourse import bass_utils

    nc = _build()

    kTs = [np.ascontiguousarray(k[b].T) for b in range(B)]
    vs = [np.ascontiguousarray(v[b]).astype(np.float16) for b in range(B)]
    in_maps = []
    for core in range(8):
        b, h = core // 2, core % 2
        in_maps.append({
            "qT": np.ascontiguousarray(q[b].T[:, h * NLOC:(h + 1) * NLOC]),
            "kT": kTs[b],
            "v": vs[b],
            "wu": np.ascontiguousarray(Wu),
            "wv": np.ascontiguousarray(Wv),
        })

    res = bass_utils.run_bass_kernel_spmd(nc, in_maps, core_ids=list(range(8)))
    LAST_RESULT = res

    out = np.empty((B, N, D), dtype=np.float32)
    for core in range(8):
        b, h = core // 2, core % 2
        out[b, h * NLOC:(h + 1) * NLOC, :] = res.results[core]["o"]
    return out


# revision 10
# speedup vs baseline: 1.3593x; 1.0141x over previous
"""Low-rank attention kernel for Trainium2, distributed over 8 NeuronCores.

Math (per batch b):
    u  = q @ Wu            [N, R]
    vp = k @ Wv            [N, R]
    S  = u @ vp.T / sqrt(R)
    out = softmax(S) @ v   [N, D]

Shapes: B=4, N=4096, D=1024, R=32.

Sharding: data-parallel over batch x row-halves -> 8 shards. Core c handles
batch b = c // 2, rows [h*2048, (h+1)*2048) with h = c % 2. Each core gets its
q-shard and the full k/v for its batch. q/k are fed pre-transposed ([D, n]
layout) so every matmul contraction lands on the partition axis with no
on-device transposes.

Per-core device kernel (all matmuls in float32r: full PE rate, ~1e-4 rel err):
  1. uT[R, 2048]  = sum_d Wu[d, :].T qT[d, :]   (K=128 d-tiles, PSUM accum)
     vpT[R, 4096] = sum_d Wv[d, :].T kT[d, :]
  2. flash-style main loop over n-chunks of 256 rows:
       for each m-tile (128 cols): scoresT[m128, n256] = vpT_tile.T @ uT_chunk
       expT = Exp(scoresT / sqrt(R))                       (ScalarE, PSUM->SBUF)
       out_acc[n128, d512] += expT_tile.T @ v_tile         (PSUM accum over m)
       sum_acc[n128, 1]    += expT_tile.T @ ones
     out = out_acc * (1 / sum_acc)   (softmax normalization folded at the end)
"""

import numpy as np

B, N, D, R = 4, 4096, 1024, 32
NLOC = N // 2            # rows per core
RSCALE = float(1.0 / np.sqrt(np.float32(R)))

N_CHUNK = 256            # rows of scores computed per PSUM round
M_TILE = 128             # contraction tile for the AV matmul
D_HALF = 512             # PSUM bank width in fp32

LAST_RESULT = None       # test.py reads exec_time_ns etc. from here


def _build():
    from concourse import bacc, mybir
    from concourse.tile import TileContext

    f32 = mybir.dt.float32
    f32r = mybir.dt.float32r
    f16 = mybir.dt.float16
    EXP = mybir.ActivationFunctionType.Exp
    COPY = mybir.ActivationFunctionType.Copy

    nc = bacc.Bacc("TRN2", target_bir_lowering=False)

    qT = nc.dram_tensor("qT", [D, NLOC], f32r, kind="ExternalInput")
    kT = nc.dram_tensor("kT", [D, N], f32r, kind="ExternalInput")
    v = nc.dram_tensor("v", [N, D], f16, kind="ExternalInput")
    wu = nc.dram_tensor("wu", [D, R], f32r, kind="ExternalInput")
    wv = nc.dram_tensor("wv", [D, R], f32r, kind="ExternalInput")
    o = nc.dram_tensor("o", [NLOC, D], f32, kind="ExternalOutput")

    DT = D // 128         # 8 d-tiles
    NQ = NLOC // 1024     # 2 column-halves of qT
    MQ = N // 1024        # 4 column-quarters of kT
    NCH = NLOC // N_CHUNK  # 8 main-loop chunks
    MT = N // M_TILE      # 32 m tiles
    VG = 8                # v row-groups of 512
    VPG = N // VG // 128  # 4 m-tiles per v group

    with TileContext(nc) as tc:
        with tc.tile_pool(name="singles", bufs=1) as singles, \
             tc.tile_pool(name="stream", bufs=20) as stream, \
             tc.tile_pool(name="vpool", bufs=VG) as vpool, \
             tc.tile_pool(name="expp", bufs=6) as expp, \
             tc.tile_pool(name="outp", bufs=3) as outp, \
             tc.tile_pool(name="rpool", bufs=4) as rpool, \
             tc.tile_pool(name="pacc", bufs=4, space="PSUM") as pacc, \
             tc.tile_pool(name="pscore", bufs=3, space="PSUM") as pscore, \
             tc.tile_pool(name="psums", bufs=1, space="PSUM") as psums:

            # ---- constants / projection weights ----
            wu_sb = singles.tile([128, DT, R], f32r, tag="wu")
            nc.sync.dma_start(out=wu_sb, in_=wu.rearrange("(t p) r -> p t r", p=128))
            wv_sb = singles.tile([128, DT, R], f32r, tag="wv")
            nc.sync.dma_start(out=wv_sb, in_=wv.rearrange("(t p) r -> p t r", p=128))
            ones = singles.tile([128, 2], f16, tag="ones")
            nc.vector.memset(ones, 1.0)

            uT = singles.tile([R, NLOC], f32r, tag="uT")
            vpT = singles.tile([R, N], f32r, tag="vpT")

            # ---- phase 1a: uT = Wu.T @ q  (per d-tile: wu_sb[:,t,:].T @ qT_t)
            def load_qt(h):
                tiles = []
                for t in range(DT):
                    tile = stream.tile([128, 1024], f32r, tag="stream",
                                       name=f"qt{h}_{t}")
                    nc.sync.dma_start(
                        out=tile, in_=qT[t * 128:(t + 1) * 128,
                                         h * 1024:(h + 1) * 1024])
                    tiles.append(tile)
                return tiles

            qt = {}
            for t, tile in enumerate(load_qt(0)):
                qt[(t, 0)] = tile
            def u_chunk(c):
                h, off = c // 2, (c % 2) * 512
                pu = pscore.tile([R, 512], f32, tag="scores", name=f"pu{c}")
                for t in range(DT):
                    nc.tensor.matmul(pu, lhsT=wu_sb[:, t, :],
                                     rhs=qt[(t, h)][:, off:off + 512],
                                     start=(t == 0), stop=(t == DT - 1))
                nc.vector.tensor_copy(out=uT[:, c * 512:(c + 1) * 512], in_=pu)

            for c in (0, 1):
                u_chunk(c)

            # ---- v tiles, interleaved with kT quarters so neither starves
            v_sb = [None] * VG

            def load_v(g):
                vt = vpool.tile([128, VPG, D], f16, tag="v", name=f"v{g}")
                nc.sync.dma_start(
                    out=vt, in_=v[g * 512:(g + 1) * 512, :].rearrange(
                        "(t p) d -> p t d", p=128))
                v_sb[g] = vt

            load_v(0)
            load_v(1)

            # ---- phase 1b: vpT = Wv.T @ k
            for qtr in range(MQ):
                kt = []
                for t in range(DT):
                    tile = stream.tile([128, 1024], f32r, tag="stream")
                    nc.sync.dma_start(
                        out=tile, in_=kT[t * 128:(t + 1) * 128,
                                         qtr * 1024:(qtr + 1) * 1024])
                    kt.append(tile)
                if qtr < 3:
                    load_v(2 + 2 * qtr)
                    load_v(3 + 2 * qtr)
                for c2 in range(2):
                    pv = pscore.tile([R, 512], f32, tag="scores")
                    for t in range(DT):
                        nc.tensor.matmul(pv, lhsT=wv_sb[:, t, :],
                                         rhs=kt[t][:, c2 * 512:c2 * 512 + 512],
                                         start=(t == 0), stop=(t == DT - 1))
                    off = qtr * 1024 + c2 * 512
                    nc.vector.tensor_copy(out=vpT[:, off:off + 512], in_=pv)

            for t, tile in enumerate(load_qt(1)):
                qt[(t, 1)] = tile
            for c in (2, 3):
                u_chunk(c)

            # ---- phase 2: flash-style scores/softmax/AV ----
            # software-pipelined: scores/exp for m-tile mt+1 are issued before
            # the AV matmuls of m-tile mt, so ScalarE exp latency hides under
            # the previous tile's AV work on the PE.
            for ch in range(NCH):
                accs = [pacc.tile([128, D_HALF], f32, tag="acc", name=f"acc{ch}_{i}")
                        for i in range(4)]
                # both sums accumulators share one bank: start=True clears
                # has_written bank-wide, so ONLY sums[0]'s first matmul carries
                # start=True (issued before any other write to the bank); the
                # cleared has_written makes sums[1]'s first start=False matmul
                # overwrite rather than accumulate stale data
                sums_t = psums.tile([128, 4], f32, tag="sums", name=f"sum{ch}")
                sums = [sums_t[:, 0:2], sums_t[:, 2:4]]

                def scores_exp(mt):
                    ps = pscore.tile([128, N_CHUNK], f32, tag="scores",
                                     name=f"ps{ch}_{mt}")
                    nc.tensor.matmul(
                        ps, lhsT=vpT[:, mt * 128:(mt + 1) * 128],
                        rhs=uT[:, ch * N_CHUNK:(ch + 1) * N_CHUNK],
                        start=True, stop=True)
                    ex = expp.tile([128, N_CHUNK], f16, tag="ex",
                                   name=f"ex{ch}_{mt}")
                    nc.scalar.activation(out=ex, in_=ps, func=EXP, scale=RSCALE)
                    return ex

                ex_q = [scores_exp(0), scores_exp(1)]
                for mt in range(MT):
                    ex = ex_q.pop(0)
                    if mt + 2 < MT:
                        ex_q.append(scores_exp(mt + 2))
                    g, tg = mt // VPG, mt % VPG
                    first, last = (mt == 0), (mt == MT - 1)
                    for j in range(2):
                        lhs = ex[:, j * 128:(j + 1) * 128]
                        nc.tensor.matmul(accs[2 * j], lhsT=lhs,
                                         rhs=v_sb[g][:, tg, 0:D_HALF],
                                         start=first, stop=last)
                        nc.tensor.matmul(accs[2 * j + 1], lhsT=lhs,
                                         rhs=v_sb[g][:, tg, D_HALF:D],
                                         start=first, stop=last)
                        nc.tensor.matmul(sums[j], lhsT=lhs, rhs=ones,
                                         start=(first and j == 0), stop=last,
                                         skip_group_check=True)
                # normalize on DVE (keeps ScalarE free for next chunk's exp)
                for j in range(2):
                    rc = rpool.tile([128, 1], f32, tag="rc", name=f"rc{ch}_{j}")
                    nc.vector.reciprocal(rc, sums[j][:, 0:1])
                    ob = outp.tile([128, D], f32, tag="ob", name=f"ob{ch}_{j}")
                    nc.vector.tensor_scalar_mul(ob[:, 0:D_HALF], accs[2 * j], rc)
                    nc.vector.tensor_scalar_mul(ob[:, D_HALF:D], accs[2 * j + 1], rc)
                    row = ch * N_CHUNK + j * 128
                    nc.sync.dma_start(out=o[row:row + 128, :], in_=ob)

    nc.finalize()
    return nc


def kernel(q, k, v, Wu, Wv):
    global LAST_RESULT
    from concourse import bass_utils

    nc = _build()

    kTs = [np.ascontiguousarray(k[b].T) for b in range(B)]
    vs = [np.ascontiguousarray(v[b]).astype(np.float16) for b in range(B)]
    in_maps = []
    for core in range(8):
        b, h = core // 2, core % 2
        in_maps.append({
            "qT": np.ascontiguousarray(q[b].T[:, h * NLOC:(h + 1) * NLOC]),
            "kT": kTs[b],
            "v": vs[b],
            "wu": np.ascontiguousarray(Wu),
            "wv": np.ascontiguousarray(Wv),
        })

    res = bass_utils.run_bass_kernel_spmd(nc, in_maps, core_ids=list(range(8)))
    LAST_RESULT = res

    out = np.empty((B, N, D), dtype=np.float32)
    for core in range(8):
        b, h = core // 2, core % 2
        out[b, h * NLOC:(h + 1) * NLOC, :] = res.results[core]["o"]
    return out
